# revision 60
# baseline (speedup 1.0000x reference)
"""Trainium2 Bass kernel for nn_BioClassifier: whitening + sequential Oja scan + readout.

Design (v2 structure + fp16 + critical-cast engine tuning):
- Jacobi-decoupled fixed-point ring (A- and B-chains independent within an
  iteration; 6 iterations) -> serial depth 3 matmul+cast pairs per iteration
  instead of 6.
- "+I" / "-Sxx" / "-I" terms folded into the matmul accumulation group as an
  extra lhsT=identity chunk, so ring PSUM->SBUF hand-offs are plain casts.
- Sign-folding: the B tile stores -B (negB) so every consumer needs no extra
  negation (ct, b1, u, ut all come out with the right sign).
- Cross-block pipelining: y0/y0t for block b+1 = base (vs W_b, computed as
  ring filler) + rank-K correction  lr*yt^T(U^T X_{b+1})  on the critical
  path; W/WT tile updates, t0t, logits, whitening all run as ring filler.
- Whitening: x shipped fp16 padded [B,896]; xc = x - mu on DVE (2x mode);
  DMA-XBAR transposes ([128,896] -> [128,7,128]) replace all PE transposes;
  xw = xc @ M^T directly (M fp16) -> XT fp16.
- ALL 16-bit tensors are fp16 (not bf16): same PE/DVE rates, 10 vs 7
  mantissa bits -> HW rel err 9.6e-3 vs 1.34e-2.  All magnitudes here are
  bounded ~1e3 << fp16 max 65504.
- The A-chain's wave-6 cast and the CU eviction run on DVE (lower
  PSUM-read latency than ACT); B-chain casts stay on ACT.
- LOOKAHEAD=2 (minimum safe): whitening becomes scheduler-ready close to
  where it is consumed, so the greedy list scheduler packs it into the
  block transitions instead of crowding ring iterations (HW-measured
  432-435us vs 448-454us at LOOKAHEAD=4; 450us at 3).
- The whitening xw matmuls are additionally READINESS-GATED per d-chunk on
  successive ring iterates (zero-cost DVE bypass ops on the xct chunks):
  they trickle through ring iterations 0-3 keeping the PE HAM-warm (the
  ring alone is ~27% PE duty, which re-throttles the clock to 1.2GHz),
  instead of being greedily drained early or stacked into the transition.
  HW-measured 425-428us vs 432-436 ungated; spreading over all 6 iterates
  over-serializes the psum accumulation staircase (477us).
All 8 cores run the identical program (scan is sequential; core 0 returned).
"""

import os
import sys
from contextlib import ExitStack, contextmanager

sys.path.insert(0, "/opt/trn_rl_repo")

import numpy as np
import ml_dtypes

import concourse.bass as bass
import concourse.mybir as mybir
from concourse.tile import TileContext
from concourse.masks import make_identity
from concourse.bass_utils import run_bass_kernel_spmd
from concourse.vector_clock import ScopedClock

LR = 1e-3
B, D, H, O = 2048, 784, 256, 10
K = 128
NBLK = B // K
DPAD = 896
DC = 7                    # 896 = 7 * 128
DH = 448                  # free-dim half for D-wide psum tiles
HC = 2
HP = 128

RING_ITERS = int(os.environ.get("RING_ITERS", "6"))
LOOKAHEAD = 2   # minimum safe; see docstring (measured optimum)
N_CORES = 8

f32 = mybir.dt.float32
# fp16 for all 16-bit tensors: same PE/DVE rates as bf16, 10 vs 7 mantissa
# bits; all magnitudes here are bounded ~1e3 << fp16 max 65504.
bf16 = mybir.dt.float16

PRI_DW = 5_000_000       # W updates, t0t, bases
PRI_OUT = 6_000_000      # logits/relu/ut
PRI_WHITEN = 8_000_000


def _install_ntff_hook():
    """The agent image's `antenv` lacks `axon_hooks`, so trace=True degrades.
    Synthesize the module and register the ctypes NTFF hook from trn_boot."""
    import types
    import antenv

    if getattr(antenv, "axon_hooks", None) is not None:
        return
    mod = types.ModuleType("antenv.axon_hooks")
    _hook_box = [None]
    mod.set_axon_ntff_profile_hook = lambda h: _hook_box.__setitem__(0, h)
    mod.get_axon_ntff_profile_hook = lambda: _hook_box[0]
    sys.modules["antenv.axon_hooks"] = mod
    antenv.axon_hooks = mod
    try:
        sys.path.insert(0, "/root/.axon_site")
        from trn_agent_boot.trn_boot import _ntff_profile_via_ctypes

        hook = _ntff_profile_via_ctypes("/opt/axon/libaxon_pjrt.so")
        if hook is not None:
            mod.set_axon_ntff_profile_hook(hook)
    except Exception:
        pass


try:
    _install_ntff_hook()
except Exception:
    pass

_drain_patched = False


def _patch_drain():
    """This walrus build only supports one sync-wait per CTRL instruction;
    split the Tile kernel-tail drain into one drain per semaphore wait."""
    global _drain_patched
    if _drain_patched:
        return

    def patched(self, tick_clock, wait_clock):
        drain_inst = self.nc.sync.drain()
        wait_clock.add_sem_waits(
            drain_inst.ins, ScopedClock({None: tick_clock.global_clock})
        )
        mi = drain_inst.ins
        si = mi.sync_info
        if si is not None and len(si.on_wait) > 1:
            waits = list(si.on_wait)
            mi.sync_info = mybir.SyncInfo(
                on_wait=[waits[0]], on_update=list(si.on_update)
            )
            for w in waits[1:]:
                d2 = self.nc.sync.drain()
                d2.ins.sync_info = mybir.SyncInfo(on_wait=[w], on_update=[])
        self.nc.all_engine_barrier()
        assert self.sems is not None
        popped = self.nc._tile_sem_poison_stack.pop()
        assert popped is self._sem_poison
        self.nc.clear_and_free_semaphores(list(self.sems.allocated().values()))
        self.nc.all_engine_barrier()

    TileContext._drain_and_barrier = patched
    _drain_patched = True


def _split_multiwait(nc, limit=1):
    """This walrus build supports only `limit` sync-waits per instruction.
    Hoist extra waits onto NoOps inserted just before, in the same engine
    stream (engines are in-order, so earlier waits are strictly safe)."""
    n_split = 0
    for f in nc.m.functions:
        for bb in f.blocks:
            insts = list(bb.instructions)
            if not any(
                i.sync_info is not None and len(i.sync_info.on_wait) > limit
                for i in insts
            ):
                continue
            new = []
            for inst in insts:
                si = inst.sync_info
                if si is not None and len(si.on_wait) > limit:
                    waits = list(si.on_wait)
                    for j, w in enumerate(waits[: len(waits) - limit]):
                        nop = mybir.InstNoOp(
                            name=f"{inst.name}-hw{j}", engine=inst.engine,
                            ins=[], outs=[],
                        )
                        nop.sync_info = mybir.SyncInfo(on_wait=[w], on_update=[])
                        new.append(nop)
                        n_split += 1
                    inst.sync_info = mybir.SyncInfo(
                        on_wait=waits[len(waits) - limit:],
                        on_update=list(si.on_update),
                    )
                new.append(inst)
            bb.instructions = new
    return n_split


def build_nc(ring_iters=RING_ITERS):
    _patch_drain()
    nc = bass.Bass()
    AT = mybir.AluOpType
    AF = mybir.ActivationFunctionType

    x_d = nc.dram_tensor("x_bf", [B, DPAD], bf16, kind="ExternalInput")
    mu_d = nc.dram_tensor("mu_bb", [128, DPAD], bf16, kind="ExternalInput")
    mt_d = nc.dram_tensor("mtb", [128, DC, DPAD], bf16, kind="ExternalInput")
    w_d = nc.dram_tensor("w0", [HP, HC, DPAD], bf16, kind="ExternalInput")
    wt_d = nc.dram_tensor("wt0", [128, DC, H], bf16, kind="ExternalInput")
    rt_d = nc.dram_tensor("rtb", [HP, HC, O], bf16, kind="ExternalInput")
    bb_d = nc.dram_tensor("b_b", [128, O], f32, kind="ExternalInput")
    out_d = nc.dram_tensor("out", [B, O], f32, kind="ExternalOutput")

    with TileContext(nc) as tc, ExitStack() as ctx:
        persist = ctx.enter_context(tc.tile_pool(name="persist", bufs=1))
        xpool = ctx.enter_context(tc.tile_pool(name="xpool", bufs=3))
        nsx = ctx.enter_context(tc.tile_pool(name="nsx", bufs=LOOKAHEAD + 2))
        blk = ctx.enter_context(tc.tile_pool(name="blk", bufs=3))
        ring = ctx.enter_context(tc.tile_pool(name="ring", bufs=4))
        # PSUM slots are bank-granular: 8 total. 5 for the ring + post-ring
        # critical chain, 3 for all filler.
        psK = ctx.enter_context(tc.tile_pool(name="psK", bufs=5, space="PSUM"))
        psF = ctx.enter_context(tc.tile_pool(name="psF", bufs=3, space="PSUM"))

        @contextmanager
        def pri(p):
            old = tc.cur_priority
            tc.cur_priority = p
            try:
                yield
            finally:
                tc.cur_priority = old

        # ---------------- constants ----------------
        idf = persist.tile([K, K], f32, tag="idf")
        make_identity(nc, idf)
        identb = persist.tile([K, K], bf16, tag="identb")
        nc.vector.tensor_copy(identb, idf)
        negidb = persist.tile([K, K], bf16, tag="negidb")
        nc.scalar.mul(negidb, idf, -1.0)
        mskf = persist.tile([K, K], f32, tag="mskf")
        nc.gpsimd.memset(mskf, LR)
        nc.gpsimd.affine_select(
            out=mskf, in_=mskf, compare_op=AT.is_gt, fill=0.0,
            base=0, pattern=[[-1, K]], channel_multiplier=1,
        )
        maskSL = persist.tile([K, K], bf16, tag="maskSL")
        nc.vector.tensor_copy(maskSL, mskf)
        negMask = persist.tile([K, K], bf16, tag="negMask")
        nc.scalar.mul(negMask, mskf, -1.0)

        # ---------------- persistent inputs ----------------
        mu_bb = persist.tile([128, DPAD], bf16, tag="mu")
        nc.sync.dma_start(out=mu_bb, in_=mu_d[:, :])
        mtb = persist.tile([128, DC, DPAD], bf16, tag="mtb")
        # per-chunk loads: the first whitening matmuls only wait for their
        # own chunk instead of the whole 1.6MB transfer (startup overlap)
        for _ic in range(DC):
            nc.sync.dma_start(out=mtb[:, _ic, :], in_=mt_d[:, _ic, :])
        Wb = persist.tile([HP, HC, DPAD], bf16, tag="Wb")
        for _hc in range(HC):
            nc.sync.dma_start(out=Wb[:, _hc, :], in_=w_d[:, _hc, :])
        WTb = persist.tile([128, DC, H], bf16, tag="WTb")
        for _ic in range(DC):
            nc.sync.dma_start(out=WTb[:, _ic, :], in_=wt_d[:, _ic, :])
        rtb = persist.tile([HP, HC, O], bf16, tag="rtb")
        nc.sync.dma_start(out=rtb, in_=rt_d[:, :, :])
        bb = persist.tile([128, O], f32, tag="bb")
        nc.sync.dma_start(out=bb, in_=bb_d[:, :])

        XTall = persist.tile([128, NBLK, DPAD], bf16, tag="XTall")
        Xall = persist.tile([128, NBLK, DC, K], bf16, tag="Xall")

        # ---------------- whitening ----------------
        def whiten(bi, gates=None):
            xraw = xpool.tile([128, DPAD], bf16, tag="xraw")
            nc.sync.dma_start(out=xraw, in_=x_d[bi * K:(bi + 1) * K, :])
            xcb = xpool.tile([128, DPAD], bf16, tag="xcb")
            nc.vector.tensor_sub(xcb, xraw, mu_bb)
            xct = xpool.tile([128, DC, K], bf16, tag="xct")
            nc.sync.dma_start_transpose(out=xct, in_=xcb)
            if gates is None:
                xs = xct
            else:
                # Readiness gates: chunk ic becomes schedulable only once
                # ring iterate gates[ic//2] exists, so these matmuls trickle
                # through the ring keeping the PE HAM-warm instead of being
                # greedily drained or stacked into the transition.
                xs = xpool.tile([128, DC, K], bf16, tag="xctg")
                for ic in range(DC):
                    nc.vector.tensor_tensor(
                        xs[:, ic, :], xct[:, ic, :],
                        gates[min(ic // 2, len(gates) - 1)], AT.bypass,
                    )
            for s in range(2):
                ps = psF.tile([K, DH], f32, tag="f")
                for ic in range(DC):
                    nc.tensor.matmul(
                        ps, xs[:, ic, :], mtb[:, ic, s * DH:(s + 1) * DH],
                        start=(ic == 0), stop=(ic == DC - 1),
                    )
                nc.scalar.copy(XTall[:, bi, s * DH:(s + 1) * DH], ps)
            nc.sync.dma_start_transpose(out=Xall[:, bi], in_=XTall[:, bi, :])
            ps2 = psF.tile([K, K], f32, tag="f")
            for ic in range(DC):
                nc.tensor.matmul(
                    ps2, Xall[:, bi, ic, :], Xall[:, bi, ic, :],
                    start=(ic == 0), stop=(ic == DC - 1),
                )
            nsxx = nsx.tile([K, K], bf16, tag="nsxx")
            nc.scalar.mul(nsxx, ps2, -1.0)
            if bi >= 1:
                # -CXX for pair (bi-1, bi): -X_{bi-1}^T X_bi
                ps3 = psF.tile([K, K], f32, tag="f")
                for ic in range(DC):
                    nc.tensor.matmul(
                        ps3, Xall[:, bi - 1, ic, :], Xall[:, bi, ic, :],
                        start=(ic == 0), stop=(ic == DC - 1),
                    )
                ncx = nsx.tile([K, K], bf16, tag="ncxx")
                nc.scalar.mul(ncx, ps3, -1.0)
                ncxx_t[bi] = ncx
            return nsxx

        nsxx_t = {}
        ncxx_t = {}
        for bi in range(min(LOOKAHEAD, NBLK)):
            nsxx_t[bi] = whiten(bi)

        # ---------------- block 0 bases ----------------
        py = psF.tile([K, H], f32, tag="f")
        for ic in range(DC):
            nc.tensor.matmul(
                py, Xall[:, 0, ic, :], WTb[:, ic, :],
                start=(ic == 0), stop=(ic == DC - 1),
            )
        y0t = blk.tile([K, H], bf16, tag="y0t")
        nc.scalar.copy(y0t, py)
        y0 = blk.tile([HP, HC, K], bf16, tag="y0")
        for hc in range(HC):
            ph = psF.tile([HP, K], f32, tag="f")
            for ic in range(DC):
                nc.tensor.matmul(
                    ph, WTb[:, ic, hc * HP:(hc + 1) * HP], Xall[:, 0, ic, :],
                    start=(ic == 0), stop=(ic == DC - 1),
                )
            nc.scalar.copy(y0[:, hc, :], ph)
        psy = psF.tile([K, K], f32, tag="f")
        for hc in range(HC):
            nc.tensor.matmul(
                psy, y0[:, hc, :], y0[:, hc, :],
                start=(hc == 0), stop=(hc == HC - 1),
            )
        syy = blk.tile([K, K], bf16, tag="syy")
        nc.scalar.copy(syy, psy)

        ytlr_prev = None
        ut_prev = None

        # ---------------- block loop ----------------
        for bi in range(NBLK):
            nsxx = nsxx_t.pop(bi)

            # ---- filler emitted before ring: scheduled into ring gaps ----
            with pri(PRI_DW + bi * 1000):
                # apply dW_{bi-1}: W tiles become W_bi
                if ytlr_prev is not None:
                    for hc in range(HC):
                        for s in range(2):
                            pw = psF.tile([HP, DH], f32, tag="f")
                            nc.tensor.matmul(
                                pw, ytlr_prev[:, hc * HP:(hc + 1) * HP],
                                ut_prev[:, s * DH:(s + 1) * DH],
                                start=True, stop=True,
                            )
                            # halve the adds so ring-critical DVE ops are
                            # blocked behind at most ~224-col filler ops
                            for hh in range(2):
                                lo = s * DH + hh * (DH // 2)
                                wsl = Wb[:, hc, lo:lo + DH // 2]
                                nc.vector.tensor_add(
                                    wsl, wsl, pw[:, hh * (DH // 2):(hh + 1) * (DH // 2)]
                                )
                    for ic in range(DC):
                        pw2 = psF.tile([128, H], f32, tag="f")
                        nc.tensor.matmul(
                            pw2, ut_prev[:, ic * K:(ic + 1) * K], ytlr_prev,
                            start=True, stop=True,
                        )
                        for hh in range(2):
                            wtsl = WTb[:, ic, hh * (H // 2):(hh + 1) * (H // 2)]
                            nc.vector.tensor_add(
                                wtsl, wtsl, pw2[:, hh * (H // 2):(hh + 1) * (H // 2)]
                            )
                # t0t_bi = y0^T W_bi  (needed at ring end for q)
                t0t = blk.tile([K, DPAD], bf16, tag="t0t")
                for s in range(2):
                    pt = psF.tile([K, DH], f32, tag="f")
                    for hc in range(HC):
                        nc.tensor.matmul(
                            pt, y0[:, hc, :], Wb[:, hc, s * DH:(s + 1) * DH],
                            start=(hc == 0), stop=(hc == HC - 1),
                        )
                    nc.scalar.copy(t0t[:, s * DH:(s + 1) * DH], pt)
                # bases for block bi+1 vs W_bi
                if bi + 1 < NBLK:
                    pb = psF.tile([K, H], f32, tag="f")
                    for ic in range(DC):
                        nc.tensor.matmul(
                            pb, Xall[:, bi + 1, ic, :], WTb[:, ic, :],
                            start=(ic == 0), stop=(ic == DC - 1),
                        )
                    y0tbase = blk.tile([K, H], f32, tag="y0tb")
                    nc.scalar.copy(y0tbase, pb)
                    y0base = blk.tile([HP, HC, K], f32, tag="y0b")
                    y0basebb = blk.tile([HP, HC, K], bf16, tag="y0bb")
                    for hc in range(HC):
                        pb2 = psF.tile([HP, K], f32, tag="f")
                        for ic in range(DC):
                            nc.tensor.matmul(
                                pb2, WTb[:, ic, hc * HP:(hc + 1) * HP],
                                Xall[:, bi + 1, ic, :],
                                start=(ic == 0), stop=(ic == DC - 1),
                            )
                        nc.scalar.copy(y0base[:, hc, :], pb2)
                        nc.vector.tensor_copy(y0basebb[:, hc, :], pb2)
                    # CT0X = y0_bi^T y0base_{bi+1}  (feeds the CU shortcut)
                    pcx = psF.tile([K, K], f32, tag="f")
                    for hc in range(HC):
                        nc.tensor.matmul(
                            pcx, y0[:, hc, :], y0basebb[:, hc, :],
                            start=(hc == 0), stop=(hc == HC - 1),
                        )
                    ct0x = blk.tile([K, K], bf16, tag="ct0x")
                    nc.scalar.copy(ct0x, pcx)

            # ---- ring (critical path) ----
            A_tiles = []
            A = identb
            nB = negidb
            CU = None
            for m in range(ring_iters):
                if m == ring_iters - 1 and bi + 1 < NBLK:
                    # CU shortcut on the PENULTIMATE iterates, emitted before
                    # the last iteration so it overlaps it (CU only feeds the
                    # lr-scaled y0 corrections: one-iteration staleness is
                    # O(lr*rho^5), sim-verified at 1.32e-2)
                    pin = psK.tile([K, K], f32, tag="kk")
                    nc.tensor.matmul(pin, identb, ncxx_t.pop(bi + 1),
                                     start=True, stop=False)
                    nc.tensor.matmul(pin, A, ct0x, start=False, stop=True)
                    minner = blk.tile([K, K], bf16, tag="minner")
                    nc.vector.tensor_copy(minner, pin)
                    pc = psK.tile([K, K], f32, tag="kk")
                    nc.tensor.matmul(pc, nB, minner, start=True, stop=True)
                    CU = blk.tile([K, K], bf16, tag="CU")
                    nc.vector.tensor_copy(CU, pc)
                    # whole correction chain on the penultimate iterates too
                    # (lr-scaled: one-iteration staleness simmed at 1.34e-2);
                    # overlaps the final ring iteration below.
                    pyl = psK.tile([K, H], f32, tag="kk")
                    nc.tensor.matmul(pyl, A, y0t, start=True, stop=True)
                    ytlr = blk.tile([K, H], bf16, tag="ytlr")
                    nc.scalar.mul(ytlr, pyl, LR)
                    pct = psK.tile([K, H], f32, tag="kk")
                    nc.tensor.matmul(pct, CU, ytlr, start=True, stop=True)
                    y0t_n = blk.tile([K, H], bf16, tag="y0t")
                    nc.vector.tensor_add(y0t_n, pct, y0tbase)
                    y0_n = blk.tile([HP, HC, K], bf16, tag="y0")
                    for hc in range(HC):
                        pcy = psK.tile([HP, K], f32, tag="kk")
                        nc.tensor.matmul(
                            pcy, ytlr[:, hc * HP:(hc + 1) * HP], CU,
                            start=True, stop=True,
                        )
                        nc.vector.tensor_add(
                            y0_n[:, hc, :], pcy, y0base[:, hc, :]
                        )
                    ps3 = psK.tile([K, K], f32, tag="kk")
                    for hc in range(HC):
                        nc.tensor.matmul(
                            ps3, y0_n[:, hc, :], y0_n[:, hc, :],
                            start=(hc == 0), stop=(hc == HC - 1),
                        )
                    syy_n = blk.tile([K, K], bf16, tag="syy")
                    nc.scalar.copy(syy_n, ps3)
                Ap, nBp = A, nB
                # Wave emission: A- and B-chain ops interleaved per dependency
                # layer so the in-order engine streams let the chains overlap.
                # wave 1: r1s = A^T syy - sxx ; z2 = syy^T A
                r1s = psK.tile([K, K], f32, tag="kk")
                nc.tensor.matmul(r1s, identb, nsxx, start=True, stop=False)
                nc.tensor.matmul(r1s, Ap, syy, start=False, stop=True)
                z2 = psK.tile([K, K], f32, tag="kk")
                nc.tensor.matmul(z2, syy, Ap, start=True, stop=True)
                # wave 2: casts (split engines)
                s1 = ring.tile([K, K], bf16, tag="s1")
                nc.vector.tensor_copy(s1, r1s)
                z2s = ring.tile([K, K], bf16, tag="z2s")
                nc.scalar.copy(z2s, z2)
                # wave 3: ct = s^T B ; g = A^T z2
                ct = psK.tile([K, K], f32, tag="kk")
                nc.tensor.matmul(ct, s1, nBp, start=True, stop=True)
                g = psK.tile([K, K], f32, tag="kk")
                nc.tensor.matmul(g, Ap, z2s, start=True, stop=True)
                # wave 4: masks (both DVE)
                nt = ring.tile([K, K], bf16, tag="nt")
                nc.vector.tensor_mul(nt, ct, maskSL)
                gm = ring.tile([K, K], bf16, tag="gm")
                nc.vector.tensor_mul(gm, g, negMask)
                # wave 5: a1 = nt^T A + I ; -B' = gm'^T(-B) - I
                a1 = psK.tile([K, K], f32, tag="kk")
                nc.tensor.matmul(a1, identb, identb, start=True, stop=False)
                nc.tensor.matmul(a1, nt, Ap, start=False, stop=True)
                b1 = psK.tile([K, K], f32, tag="kk")
                nc.tensor.matmul(b1, identb, negidb, start=True, stop=False)
                nc.tensor.matmul(b1, gm, nBp, start=False, stop=True)
                # wave 6: output casts (split engines)
                A = ring.tile([K, K], bf16, tag="A")
                nc.vector.tensor_copy(A, a1)
                A_tiles.append(A)
                nB = ring.tile([K, K], bf16, tag="nB")
                nc.scalar.copy(nB, b1)

            with pri(PRI_WHITEN + bi * 1000):
                if bi + LOOKAHEAD < NBLK:
                    nsxx_t[bi + LOOKAHEAD] = whiten(
                        bi + LOOKAHEAD, gates=A_tiles[:4]
                    )

            # ---- post-ring ----
            # feats path (not on critical path)
            with pri(PRI_OUT + bi * 1000):
                relu_y = blk.tile([HP, HC, K], bf16, tag="relu")
                for hc in range(HC):
                    ph2 = psF.tile([HP, K], f32, tag="f")
                    nc.tensor.matmul(
                        ph2, y0t[:, hc * HP:(hc + 1) * HP], A,
                        start=True, stop=True,
                    )
                    nc.scalar.activation(relu_y[:, hc, :], ph2, AF.Relu)
                lg = psF.tile([K, O], f32, tag="f")
                for hc in range(HC):
                    nc.tensor.matmul(
                        lg, relu_y[:, hc, :], rtb[:, hc, :],
                        start=(hc == 0), stop=(hc == HC - 1),
                    )
                lgs = blk.tile([K, O], f32, tag="lg")
                nc.vector.tensor_add(lgs, lg, bb)
                nc.sync.dma_start(out=out_d[bi * K:(bi + 1) * K, :], in_=lgs)

            if bi + 1 >= NBLK:
                break

            # q' = A^T t0t - XT ; ut = (-B)^T q'  (only feed dW: filler)
            with pri(PRI_DW + bi * 1000 + 500):
                q = blk.tile([K, DPAD], bf16, tag="q")
                for s in range(2):
                    pq = psF.tile([K, DH], f32, tag="f")
                    nc.tensor.matmul(
                        pq, A, t0t[:, s * DH:(s + 1) * DH], start=True, stop=True
                    )
                    nc.vector.tensor_sub(
                        q[:, s * DH:(s + 1) * DH], pq,
                        XTall[:, bi, s * DH:(s + 1) * DH],
                    )
                ut = blk.tile([K, DPAD], bf16, tag="ut")
                for s in range(2):
                    pt2 = psF.tile([K, DH], f32, tag="f")
                    nc.tensor.matmul(
                        pt2, nB, q[:, s * DH:(s + 1) * DH],
                        start=True, stop=True,
                    )
                    nc.scalar.copy(ut[:, s * DH:(s + 1) * DH], pt2)

            y0, y0t, syy = y0_n, y0t_n, syy_n
            ytlr_prev, ut_prev = ytlr, ut

    _split_multiwait(nc)
    return nc


def prep_inputs(x, whiten_mean, whiten_mat, oja_W, readout_W, readout_b):
    """Host-side layout/dtype prep (padding, transposes, casts only)."""
    bf = np.float16
    x = np.asarray(x, dtype=np.float32)
    mu = np.asarray(whiten_mean, dtype=np.float32)
    M = np.asarray(whiten_mat, dtype=np.float32)
    W0 = np.asarray(oja_W, dtype=np.float32)
    R = np.asarray(readout_W, dtype=np.float32)
    rb = np.asarray(readout_b, dtype=np.float32)

    x_bf = np.zeros((B, DPAD), dtype=bf)
    x_bf[:, :D] = x.astype(bf)
    mu_bb = np.zeros((128, DPAD), dtype=bf)
    mu_bb[:, :D] = np.broadcast_to(mu.astype(bf)[None, :], (128, D))
    # mtb[p, ic, dout] = M[dout, ic*128+p]  (zero-padded)
    mtb = np.zeros((128, DC, DPAD), dtype=bf)
    mt = M.T.astype(bf)  # [e, dout]
    for ic in range(DC):
        lo, hi = ic * 128, min((ic + 1) * 128, D)
        mtb[: hi - lo, ic, :D] = mt[lo:hi, :]
    w0 = np.zeros((HP, HC, DPAD), dtype=bf)
    w0[:, :, :D] = W0.reshape(HC, HP, D).transpose(1, 0, 2).astype(bf)
    wt0 = np.zeros((128, DC, H), dtype=bf)
    wtf = W0.T.astype(bf)  # [d, h]
    for ic in range(DC):
        lo, hi = ic * 128, min((ic + 1) * 128, D)
        wt0[: hi - lo, ic, :] = wtf[lo:hi, :]
    rtb = np.ascontiguousarray(
        R.T.reshape(HC, HP, O).transpose(1, 0, 2).astype(bf)
    )
    b_b = np.broadcast_to(rb[None, :], (128, O)).copy()
    return {
        "x_bf": x_bf, "mu_bb": mu_bb, "mtb": mtb, "w0": w0, "wt0": wt0,
        "rtb": rtb, "b_b": b_b,
    }


_cached_nc = None


def _get_nc():
    global _cached_nc
    if _cached_nc is None:
        _cached_nc = build_nc()
    return _cached_nc


def kernel(x, whiten_mean, whiten_mat, oja_W, readout_W, readout_b, **run_kwargs):
    nc = _get_nc()
    ins = prep_inputs(x, whiten_mean, whiten_mat, oja_W, readout_W, readout_b)
    res = run_bass_kernel_spmd(
        nc, [ins] * N_CORES, core_ids=list(range(N_CORES)), **run_kwargs
    )
    out = res.results[0]["out"]
    if run_kwargs:
        kernel.last_result = res
    return out



# revision 61
# speedup vs baseline: 1.0133x; 1.0133x over previous
"""Trainium2 Bass kernel for nn_BioClassifier: whitening + sequential Oja scan + readout.

Design (v2 structure + fp16 + critical-cast engine tuning):
- Jacobi-decoupled fixed-point ring (A- and B-chains independent within an
  iteration; 6 iterations) -> serial depth 3 matmul+cast pairs per iteration
  instead of 6.
- "+I" / "-Sxx" / "-I" terms folded into the matmul accumulation group as an
  extra lhsT=identity chunk, so ring PSUM->SBUF hand-offs are plain casts.
- Sign-folding: the B tile stores -B (negB) so every consumer needs no extra
  negation (ct, b1, u, ut all come out with the right sign).
- Cross-block pipelining: y0/y0t for block b+1 = base (vs W_b, computed as
  ring filler) + rank-K correction  lr*yt^T(U^T X_{b+1})  on the critical
  path; W/WT tile updates, t0t, logits, whitening all run as ring filler.
- Whitening: x shipped fp16 padded [B,896]; xc = x - mu on DVE (2x mode);
  DMA-XBAR transposes ([128,896] -> [128,7,128]) replace all PE transposes;
  xw = xc @ M^T directly (M fp16) -> XT fp16.
- ALL 16-bit tensors are fp16 (not bf16): same PE/DVE rates, 10 vs 7
  mantissa bits -> HW rel err 9.6e-3 vs 1.34e-2.  All magnitudes here are
  bounded ~1e3 << fp16 max 65504.
- The A-chain's wave-6 cast and the CU eviction run on DVE (lower
  PSUM-read latency than ACT); B-chain casts stay on ACT.
- LOOKAHEAD=2 (minimum safe): whitening becomes scheduler-ready close to
  where it is consumed, so the greedy list scheduler packs it into the
  block transitions instead of crowding ring iterations (HW-measured
  432-435us vs 448-454us at LOOKAHEAD=4; 450us at 3).
- The whitening xw matmuls are additionally READINESS-GATED per d-chunk on
  successive ring iterates (zero-cost DVE bypass ops on the xct chunks):
  they trickle through ring iterations 0-3 keeping the PE HAM-warm (the
  ring alone is ~27% PE duty, which re-throttles the clock to 1.2GHz),
  instead of being greedily drained early or stacked into the transition.
  HW-measured 425-428us vs 432-436 ungated; spreading over all 6 iterates
  over-serializes the psum accumulation staircase (477us).
All 8 cores run the identical program (scan is sequential; core 0 returned).
"""

import os
import sys
from contextlib import ExitStack, contextmanager

sys.path.insert(0, "/opt/trn_rl_repo")

import numpy as np
import ml_dtypes

import concourse.bass as bass
import concourse.mybir as mybir
from concourse.tile import TileContext
from concourse.masks import make_identity
from concourse.bass_utils import run_bass_kernel_spmd
from concourse.vector_clock import ScopedClock

LR = 1e-3
B, D, H, O = 2048, 784, 256, 10
K = 128
NBLK = B // K
DPAD = 896
DC = 7                    # 896 = 7 * 128
DH = 448                  # free-dim half for D-wide psum tiles
HC = 2
HP = 128

RING_ITERS = int(os.environ.get("RING_ITERS", "6"))
LOOKAHEAD = 2   # minimum safe; see docstring (measured optimum)
N_CORES = 8

f32 = mybir.dt.float32
# fp16 for all 16-bit tensors: same PE/DVE rates as bf16, 10 vs 7 mantissa
# bits; all magnitudes here are bounded ~1e3 << fp16 max 65504.
bf16 = mybir.dt.float16

PRI_DW = 5_000_000       # W updates, t0t, bases
PRI_OUT = 6_000_000      # logits/relu/ut
PRI_WHITEN = 8_000_000


def _install_ntff_hook():
    """The agent image's `antenv` lacks `axon_hooks`, so trace=True degrades.
    Synthesize the module and register the ctypes NTFF hook from trn_boot."""
    import types
    import antenv

    if getattr(antenv, "axon_hooks", None) is not None:
        return
    mod = types.ModuleType("antenv.axon_hooks")
    _hook_box = [None]
    mod.set_axon_ntff_profile_hook = lambda h: _hook_box.__setitem__(0, h)
    mod.get_axon_ntff_profile_hook = lambda: _hook_box[0]
    sys.modules["antenv.axon_hooks"] = mod
    antenv.axon_hooks = mod
    try:
        sys.path.insert(0, "/root/.axon_site")
        from trn_agent_boot.trn_boot import _ntff_profile_via_ctypes

        hook = _ntff_profile_via_ctypes("/opt/axon/libaxon_pjrt.so")
        if hook is not None:
            mod.set_axon_ntff_profile_hook(hook)
    except Exception:
        pass


try:
    _install_ntff_hook()
except Exception:
    pass

_drain_patched = False


def _patch_drain():
    """This walrus build only supports one sync-wait per CTRL instruction;
    split the Tile kernel-tail drain into one drain per semaphore wait."""
    global _drain_patched
    if _drain_patched:
        return

    def patched(self, tick_clock, wait_clock):
        drain_inst = self.nc.sync.drain()
        wait_clock.add_sem_waits(
            drain_inst.ins, ScopedClock({None: tick_clock.global_clock})
        )
        mi = drain_inst.ins
        si = mi.sync_info
        if si is not None and len(si.on_wait) > 1:
            waits = list(si.on_wait)
            mi.sync_info = mybir.SyncInfo(
                on_wait=[waits[0]], on_update=list(si.on_update)
            )
            for w in waits[1:]:
                d2 = self.nc.sync.drain()
                d2.ins.sync_info = mybir.SyncInfo(on_wait=[w], on_update=[])
        self.nc.all_engine_barrier()
        assert self.sems is not None
        popped = self.nc._tile_sem_poison_stack.pop()
        assert popped is self._sem_poison
        self.nc.clear_and_free_semaphores(list(self.sems.allocated().values()))
        self.nc.all_engine_barrier()

    TileContext._drain_and_barrier = patched
    _drain_patched = True


def _split_multiwait(nc, limit=1):
    """This walrus build supports only `limit` sync-waits per instruction.
    Hoist extra waits onto NoOps inserted just before, in the same engine
    stream (engines are in-order, so earlier waits are strictly safe)."""
    n_split = 0
    for f in nc.m.functions:
        for bb in f.blocks:
            insts = list(bb.instructions)
            if not any(
                i.sync_info is not None and len(i.sync_info.on_wait) > limit
                for i in insts
            ):
                continue
            new = []
            for inst in insts:
                si = inst.sync_info
                if si is not None and len(si.on_wait) > limit:
                    waits = list(si.on_wait)
                    for j, w in enumerate(waits[: len(waits) - limit]):
                        nop = mybir.InstNoOp(
                            name=f"{inst.name}-hw{j}", engine=inst.engine,
                            ins=[], outs=[],
                        )
                        nop.sync_info = mybir.SyncInfo(on_wait=[w], on_update=[])
                        new.append(nop)
                        n_split += 1
                    inst.sync_info = mybir.SyncInfo(
                        on_wait=waits[len(waits) - limit:],
                        on_update=list(si.on_update),
                    )
                new.append(inst)
            bb.instructions = new
    return n_split


def build_nc(ring_iters=RING_ITERS):
    _patch_drain()
    nc = bass.Bass()
    AT = mybir.AluOpType
    AF = mybir.ActivationFunctionType

    x_d = nc.dram_tensor("x_bf", [B, DPAD], bf16, kind="ExternalInput")
    mu_d = nc.dram_tensor("mu_bb", [128, DPAD], bf16, kind="ExternalInput")
    mt_d = nc.dram_tensor("mtb", [128, DC, DPAD], bf16, kind="ExternalInput")
    w_d = nc.dram_tensor("w0", [HP, HC, DPAD], bf16, kind="ExternalInput")
    wt_d = nc.dram_tensor("wt0", [128, DC, H], bf16, kind="ExternalInput")
    rt_d = nc.dram_tensor("rtb", [HP, HC, O], bf16, kind="ExternalInput")
    bb_d = nc.dram_tensor("b_b", [128, O], f32, kind="ExternalInput")
    out_d = nc.dram_tensor("out", [B, O], f32, kind="ExternalOutput")

    with TileContext(nc) as tc, ExitStack() as ctx:
        persist = ctx.enter_context(tc.tile_pool(name="persist", bufs=1))
        xpool = ctx.enter_context(tc.tile_pool(name="xpool", bufs=3))
        nsx = ctx.enter_context(tc.tile_pool(name="nsx", bufs=LOOKAHEAD + 2))
        blk = ctx.enter_context(tc.tile_pool(name="blk", bufs=3))
        ring = ctx.enter_context(tc.tile_pool(name="ring", bufs=4))
        # PSUM slots are bank-granular: 8 total. 5 for the ring + post-ring
        # critical chain, 3 for all filler.
        psK = ctx.enter_context(tc.tile_pool(name="psK", bufs=5, space="PSUM"))
        psF = ctx.enter_context(tc.tile_pool(name="psF", bufs=3, space="PSUM"))

        @contextmanager
        def pri(p):
            old = tc.cur_priority
            tc.cur_priority = p
            try:
                yield
            finally:
                tc.cur_priority = old

        # ---------------- constants ----------------
        idf = persist.tile([K, K], f32, tag="idf")
        make_identity(nc, idf)
        identb = persist.tile([K, K], bf16, tag="identb")
        nc.vector.tensor_copy(identb, idf)
        negidb = persist.tile([K, K], bf16, tag="negidb")
        nc.scalar.mul(negidb, idf, -1.0)
        mskf = persist.tile([K, K], f32, tag="mskf")
        nc.gpsimd.memset(mskf, LR)
        nc.gpsimd.affine_select(
            out=mskf, in_=mskf, compare_op=AT.is_gt, fill=0.0,
            base=0, pattern=[[-1, K]], channel_multiplier=1,
        )
        maskSL = persist.tile([K, K], bf16, tag="maskSL")
        nc.vector.tensor_copy(maskSL, mskf)
        negMask = persist.tile([K, K], bf16, tag="negMask")
        nc.scalar.mul(negMask, mskf, -1.0)

        # ---------------- persistent inputs ----------------
        mu_bb = persist.tile([128, DPAD], bf16, tag="mu")
        nc.sync.dma_start(out=mu_bb, in_=mu_d[:, :])
        mtb = persist.tile([128, DC, DPAD], bf16, tag="mtb")
        nc.sync.dma_start(out=mtb, in_=mt_d[:, :, :])
        Wb = persist.tile([HP, HC, DPAD], bf16, tag="Wb")
        nc.sync.dma_start(out=Wb, in_=w_d[:, :, :])
        WTb = persist.tile([128, DC, H], bf16, tag="WTb")
        nc.sync.dma_start(out=WTb, in_=wt_d[:, :, :])
        rtb = persist.tile([HP, HC, O], bf16, tag="rtb")
        nc.sync.dma_start(out=rtb, in_=rt_d[:, :, :])
        bb = persist.tile([128, O], f32, tag="bb")
        nc.sync.dma_start(out=bb, in_=bb_d[:, :])

        XTall = persist.tile([128, NBLK, DPAD], bf16, tag="XTall")
        Xall = persist.tile([128, NBLK, DC, K], bf16, tag="Xall")

        # ---------------- whitening ----------------
        def whiten(bi, gates=None):
            xraw = xpool.tile([128, DPAD], bf16, tag="xraw")
            nc.sync.dma_start(out=xraw, in_=x_d[bi * K:(bi + 1) * K, :])
            xcb = xpool.tile([128, DPAD], bf16, tag="xcb")
            nc.vector.tensor_sub(xcb, xraw, mu_bb)
            xct = xpool.tile([128, DC, K], bf16, tag="xct")
            nc.sync.dma_start_transpose(out=xct, in_=xcb)
            if gates is None:
                xs = xct
            else:
                # Readiness gates: chunk ic becomes schedulable only once
                # ring iterate gates[ic//2] exists, so these matmuls trickle
                # through the ring keeping the PE HAM-warm instead of being
                # greedily drained or stacked into the transition.
                xs = xpool.tile([128, DC, K], bf16, tag="xctg")
                for ic in range(DC):
                    nc.vector.tensor_tensor(
                        xs[:, ic, :], xct[:, ic, :],
                        gates[min(ic // 2, len(gates) - 1)], AT.bypass,
                    )
            for s in range(2):
                ps = psF.tile([K, DH], f32, tag="f")
                for ic in range(DC):
                    nc.tensor.matmul(
                        ps, xs[:, ic, :], mtb[:, ic, s * DH:(s + 1) * DH],
                        start=(ic == 0), stop=(ic == DC - 1),
                    )
                nc.scalar.copy(XTall[:, bi, s * DH:(s + 1) * DH], ps)
            nc.sync.dma_start_transpose(out=Xall[:, bi], in_=XTall[:, bi, :])
            ps2 = psF.tile([K, K], f32, tag="f")
            for ic in range(DC):
                nc.tensor.matmul(
                    ps2, Xall[:, bi, ic, :], Xall[:, bi, ic, :],
                    start=(ic == 0), stop=(ic == DC - 1),
                )
            nsxx = nsx.tile([K, K], bf16, tag="nsxx")
            nc.scalar.mul(nsxx, ps2, -1.0)
            if bi >= 1:
                # -CXX for pair (bi-1, bi): -X_{bi-1}^T X_bi
                ps3 = psF.tile([K, K], f32, tag="f")
                for ic in range(DC):
                    nc.tensor.matmul(
                        ps3, Xall[:, bi - 1, ic, :], Xall[:, bi, ic, :],
                        start=(ic == 0), stop=(ic == DC - 1),
                    )
                ncx = nsx.tile([K, K], bf16, tag="ncxx")
                nc.scalar.mul(ncx, ps3, -1.0)
                ncxx_t[bi] = ncx
            return nsxx

        nsxx_t = {}
        ncxx_t = {}
        for bi in range(min(LOOKAHEAD, NBLK)):
            nsxx_t[bi] = whiten(bi)

        # ---------------- block 0 bases ----------------
        py = psF.tile([K, H], f32, tag="f")
        for ic in range(DC):
            nc.tensor.matmul(
                py, Xall[:, 0, ic, :], WTb[:, ic, :],
                start=(ic == 0), stop=(ic == DC - 1),
            )
        y0t = blk.tile([K, H], bf16, tag="y0t")
        nc.scalar.copy(y0t, py)
        y0 = blk.tile([HP, HC, K], bf16, tag="y0")
        for hc in range(HC):
            ph = psF.tile([HP, K], f32, tag="f")
            for ic in range(DC):
                nc.tensor.matmul(
                    ph, WTb[:, ic, hc * HP:(hc + 1) * HP], Xall[:, 0, ic, :],
                    start=(ic == 0), stop=(ic == DC - 1),
                )
            nc.scalar.copy(y0[:, hc, :], ph)
        psy = psF.tile([K, K], f32, tag="f")
        for hc in range(HC):
            nc.tensor.matmul(
                psy, y0[:, hc, :], y0[:, hc, :],
                start=(hc == 0), stop=(hc == HC - 1),
            )
        syy = blk.tile([K, K], bf16, tag="syy")
        nc.scalar.copy(syy, psy)

        ytlr_prev = None
        ut_prev = None

        # ---------------- block loop ----------------
        for bi in range(NBLK):
            nsxx = nsxx_t.pop(bi)

            # ---- filler emitted before ring: scheduled into ring gaps ----
            with pri(PRI_DW + bi * 1000):
                # apply dW_{bi-1}: W tiles become W_bi
                if ytlr_prev is not None:
                    for hc in range(HC):
                        for s in range(2):
                            pw = psF.tile([HP, DH], f32, tag="f")
                            nc.tensor.matmul(
                                pw, ytlr_prev[:, hc * HP:(hc + 1) * HP],
                                ut_prev[:, s * DH:(s + 1) * DH],
                                start=True, stop=True,
                            )
                            # halve the adds so ring-critical DVE ops are
                            # blocked behind at most ~224-col filler ops
                            for hh in range(2):
                                lo = s * DH + hh * (DH // 2)
                                wsl = Wb[:, hc, lo:lo + DH // 2]
                                nc.vector.tensor_add(
                                    wsl, wsl, pw[:, hh * (DH // 2):(hh + 1) * (DH // 2)]
                                )
                    for ic in range(DC):
                        pw2 = psF.tile([128, H], f32, tag="f")
                        nc.tensor.matmul(
                            pw2, ut_prev[:, ic * K:(ic + 1) * K], ytlr_prev,
                            start=True, stop=True,
                        )
                        for hh in range(2):
                            wtsl = WTb[:, ic, hh * (H // 2):(hh + 1) * (H // 2)]
                            nc.vector.tensor_add(
                                wtsl, wtsl, pw2[:, hh * (H // 2):(hh + 1) * (H // 2)]
                            )
                # t0t_bi = y0^T W_bi  (needed at ring end for q)
                t0t = blk.tile([K, DPAD], bf16, tag="t0t")
                for s in range(2):
                    pt = psF.tile([K, DH], f32, tag="f")
                    for hc in range(HC):
                        nc.tensor.matmul(
                            pt, y0[:, hc, :], Wb[:, hc, s * DH:(s + 1) * DH],
                            start=(hc == 0), stop=(hc == HC - 1),
                        )
                    nc.scalar.copy(t0t[:, s * DH:(s + 1) * DH], pt)
                # bases for block bi+1 vs W_bi
                if bi + 1 < NBLK:
                    pb = psF.tile([K, H], f32, tag="f")
                    for ic in range(DC):
                        nc.tensor.matmul(
                            pb, Xall[:, bi + 1, ic, :], WTb[:, ic, :],
                            start=(ic == 0), stop=(ic == DC - 1),
                        )
                    y0tbase = blk.tile([K, H], f32, tag="y0tb")
                    nc.scalar.copy(y0tbase, pb)
                    y0base = blk.tile([HP, HC, K], f32, tag="y0b")
                    y0basebb = blk.tile([HP, HC, K], bf16, tag="y0bb")
                    for hc in range(HC):
                        pb2 = psF.tile([HP, K], f32, tag="f")
                        for ic in range(DC):
                            nc.tensor.matmul(
                                pb2, WTb[:, ic, hc * HP:(hc + 1) * HP],
                                Xall[:, bi + 1, ic, :],
                                start=(ic == 0), stop=(ic == DC - 1),
                            )
                        nc.scalar.copy(y0base[:, hc, :], pb2)
                        nc.vector.tensor_copy(y0basebb[:, hc, :], pb2)
                    # CT0X = y0_bi^T y0base_{bi+1}  (feeds the CU shortcut)
                    pcx = psF.tile([K, K], f32, tag="f")
                    for hc in range(HC):
                        nc.tensor.matmul(
                            pcx, y0[:, hc, :], y0basebb[:, hc, :],
                            start=(hc == 0), stop=(hc == HC - 1),
                        )
                    ct0x = blk.tile([K, K], bf16, tag="ct0x")
                    nc.scalar.copy(ct0x, pcx)

            # ---- ring (critical path) ----
            A_tiles = []
            A = identb
            nB = negidb
            CU = None
            for m in range(ring_iters):
                if m == ring_iters - 1 and bi + 1 < NBLK:
                    # CU shortcut on the PENULTIMATE iterates, emitted before
                    # the last iteration so it overlaps it (CU only feeds the
                    # lr-scaled y0 corrections: one-iteration staleness is
                    # O(lr*rho^5), sim-verified at 1.32e-2)
                    pin = psK.tile([K, K], f32, tag="kk")
                    nc.tensor.matmul(pin, identb, ncxx_t.pop(bi + 1),
                                     start=True, stop=False)
                    nc.tensor.matmul(pin, A, ct0x, start=False, stop=True)
                    minner = blk.tile([K, K], bf16, tag="minner")
                    nc.vector.tensor_copy(minner, pin)
                    pc = psK.tile([K, K], f32, tag="kk")
                    nc.tensor.matmul(pc, nB, minner, start=True, stop=True)
                    CU = blk.tile([K, K], bf16, tag="CU")
                    nc.vector.tensor_copy(CU, pc)
                    # whole correction chain on the penultimate iterates too
                    # (lr-scaled: one-iteration staleness simmed at 1.34e-2);
                    # overlaps the final ring iteration below.
                    pyl = psK.tile([K, H], f32, tag="kk")
                    nc.tensor.matmul(pyl, A, y0t, start=True, stop=True)
                    ytlr = blk.tile([K, H], bf16, tag="ytlr")
                    nc.scalar.mul(ytlr, pyl, LR)
                    pct = psK.tile([K, H], f32, tag="kk")
                    nc.tensor.matmul(pct, CU, ytlr, start=True, stop=True)
                    y0t_n = blk.tile([K, H], bf16, tag="y0t")
                    nc.vector.tensor_add(y0t_n, pct, y0tbase)
                    y0_n = blk.tile([HP, HC, K], bf16, tag="y0")
                    for hc in range(HC):
                        pcy = psK.tile([HP, K], f32, tag="kk")
                        nc.tensor.matmul(
                            pcy, ytlr[:, hc * HP:(hc + 1) * HP], CU,
                            start=True, stop=True,
                        )
                        nc.vector.tensor_add(
                            y0_n[:, hc, :], pcy, y0base[:, hc, :]
                        )
                    ps3 = psK.tile([K, K], f32, tag="kk")
                    for hc in range(HC):
                        nc.tensor.matmul(
                            ps3, y0_n[:, hc, :], y0_n[:, hc, :],
                            start=(hc == 0), stop=(hc == HC - 1),
                        )
                    syy_n = blk.tile([K, K], bf16, tag="syy")
                    nc.scalar.copy(syy_n, ps3)
                Ap, nBp = A, nB
                # Wave emission: A- and B-chain ops interleaved per dependency
                # layer so the in-order engine streams let the chains overlap.
                # wave 1: r1s = A^T syy - sxx ; z2 = syy^T A
                r1s = psK.tile([K, K], f32, tag="kk")
                nc.tensor.matmul(r1s, identb, nsxx, start=True, stop=False)
                nc.tensor.matmul(r1s, Ap, syy, start=False, stop=True)
                z2 = psK.tile([K, K], f32, tag="kk")
                nc.tensor.matmul(z2, syy, Ap, start=True, stop=True)
                # wave 2: casts (split engines)
                s1 = ring.tile([K, K], bf16, tag="s1")
                nc.vector.tensor_copy(s1, r1s)
                z2s = ring.tile([K, K], bf16, tag="z2s")
                nc.scalar.copy(z2s, z2)
                # wave 3: ct = s^T B ; g = A^T z2
                ct = psK.tile([K, K], f32, tag="kk")
                nc.tensor.matmul(ct, s1, nBp, start=True, stop=True)
                g = psK.tile([K, K], f32, tag="kk")
                nc.tensor.matmul(g, Ap, z2s, start=True, stop=True)
                # wave 4: masks (both DVE)
                nt = ring.tile([K, K], bf16, tag="nt")
                nc.vector.tensor_mul(nt, ct, maskSL)
                gm = ring.tile([K, K], bf16, tag="gm")
                nc.vector.tensor_mul(gm, g, negMask)
                # wave 5: a1 = nt^T A + I ; -B' = gm'^T(-B) - I
                a1 = psK.tile([K, K], f32, tag="kk")
                nc.tensor.matmul(a1, identb, identb, start=True, stop=False)
                nc.tensor.matmul(a1, nt, Ap, start=False, stop=True)
                b1 = psK.tile([K, K], f32, tag="kk")
                nc.tensor.matmul(b1, identb, negidb, start=True, stop=False)
                nc.tensor.matmul(b1, gm, nBp, start=False, stop=True)
                # wave 6: output casts (split engines)
                A = ring.tile([K, K], bf16, tag="A")
                nc.vector.tensor_copy(A, a1)
                A_tiles.append(A)
                nB = ring.tile([K, K], bf16, tag="nB")
                nc.scalar.copy(nB, b1)

            with pri(PRI_WHITEN + bi * 1000):
                if bi + LOOKAHEAD < NBLK:
                    nsxx_t[bi + LOOKAHEAD] = whiten(
                        bi + LOOKAHEAD, gates=A_tiles[:4]
                    )

            # ---- post-ring ----
            # feats path (not on critical path)
            with pri(PRI_OUT + bi * 1000):
                relu_y = blk.tile([HP, HC, K], bf16, tag="relu")
                for hc in range(HC):
                    ph2 = psF.tile([HP, K], f32, tag="f")
                    nc.tensor.matmul(
                        ph2, y0t[:, hc * HP:(hc + 1) * HP], A,
                        start=True, stop=True,
                    )
                    nc.scalar.activation(relu_y[:, hc, :], ph2, AF.Relu)
                lg = psF.tile([K, O], f32, tag="f")
                for hc in range(HC):
                    nc.tensor.matmul(
                        lg, relu_y[:, hc, :], rtb[:, hc, :],
                        start=(hc == 0), stop=(hc == HC - 1),
                    )
                lgs = blk.tile([K, O], f32, tag="lg")
                nc.vector.tensor_add(lgs, lg, bb)
                nc.sync.dma_start(out=out_d[bi * K:(bi + 1) * K, :], in_=lgs)

            if bi + 1 >= NBLK:
                break

            # q' = A^T t0t - XT ; ut = (-B)^T q'  (only feed dW: filler)
            with pri(PRI_DW + bi * 1000 + 500):
                q = blk.tile([K, DPAD], bf16, tag="q")
                for s in range(2):
                    pq = psF.tile([K, DH], f32, tag="f")
                    nc.tensor.matmul(
                        pq, A, t0t[:, s * DH:(s + 1) * DH], start=True, stop=True
                    )
                    nc.vector.tensor_sub(
                        q[:, s * DH:(s + 1) * DH], pq,
                        XTall[:, bi, s * DH:(s + 1) * DH],
                    )
                ut = blk.tile([K, DPAD], bf16, tag="ut")
                for s in range(2):
                    pt2 = psF.tile([K, DH], f32, tag="f")
                    nc.tensor.matmul(
                        pt2, nB, q[:, s * DH:(s + 1) * DH],
                        start=True, stop=True,
                    )
                    nc.scalar.copy(ut[:, s * DH:(s + 1) * DH], pt2)

            y0, y0t, syy = y0_n, y0t_n, syy_n
            ytlr_prev, ut_prev = ytlr, ut

    _split_multiwait(nc)
    return nc


def prep_inputs(x, whiten_mean, whiten_mat, oja_W, readout_W, readout_b):
    """Host-side layout/dtype prep (padding, transposes, casts only)."""
    bf = np.float16
    x = np.asarray(x, dtype=np.float32)
    mu = np.asarray(whiten_mean, dtype=np.float32)
    M = np.asarray(whiten_mat, dtype=np.float32)
    W0 = np.asarray(oja_W, dtype=np.float32)
    R = np.asarray(readout_W, dtype=np.float32)
    rb = np.asarray(readout_b, dtype=np.float32)

    x_bf = np.zeros((B, DPAD), dtype=bf)
    x_bf[:, :D] = x.astype(bf)
    mu_bb = np.zeros((128, DPAD), dtype=bf)
    mu_bb[:, :D] = np.broadcast_to(mu.astype(bf)[None, :], (128, D))
    # mtb[p, ic, dout] = M[dout, ic*128+p]  (zero-padded)
    mtb = np.zeros((128, DC, DPAD), dtype=bf)
    mt = M.T.astype(bf)  # [e, dout]
    for ic in range(DC):
        lo, hi = ic * 128, min((ic + 1) * 128, D)
        mtb[: hi - lo, ic, :D] = mt[lo:hi, :]
    w0 = np.zeros((HP, HC, DPAD), dtype=bf)
    w0[:, :, :D] = W0.reshape(HC, HP, D).transpose(1, 0, 2).astype(bf)
    wt0 = np.zeros((128, DC, H), dtype=bf)
    wtf = W0.T.astype(bf)  # [d, h]
    for ic in range(DC):
        lo, hi = ic * 128, min((ic + 1) * 128, D)
        wt0[: hi - lo, ic, :] = wtf[lo:hi, :]
    rtb = np.ascontiguousarray(
        R.T.reshape(HC, HP, O).transpose(1, 0, 2).astype(bf)
    )
    b_b = np.broadcast_to(rb[None, :], (128, O)).copy()
    return {
        "x_bf": x_bf, "mu_bb": mu_bb, "mtb": mtb, "w0": w0, "wt0": wt0,
        "rtb": rtb, "b_b": b_b,
    }


_cached_nc = None


def _get_nc():
    global _cached_nc
    if _cached_nc is None:
        _cached_nc = build_nc()
    return _cached_nc


def kernel(x, whiten_mean, whiten_mat, oja_W, readout_W, readout_b, **run_kwargs):
    nc = _get_nc()
    ins = prep_inputs(x, whiten_mean, whiten_mat, oja_W, readout_W, readout_b)
    res = run_bass_kernel_spmd(
        nc, [ins] * N_CORES, core_ids=list(range(N_CORES)), **run_kwargs
    )
    out = res.results[0]["out"]
    if run_kwargs:
        kernel.last_result = res
    return out

